# revision 11
# baseline (speedup 1.0000x reference)
"""Head-sharded (tensor-parallel) causal attention block for 8 NeuronCores, v2.

Model: B=2, S=2048, D=1024, H=16 heads (HD=64). Each core owns 2 heads
(128 features) of the QKV projections and attention, computes a partial
output projection (o_shard @ ow_shard), and the host sums the 8 partials
and adds the output bias.

Math shortcuts (exact):
  - K bias dropped: its score contribution (q+qb)@kb is constant over keys,
    so softmax cancels it.
  - V bias folded into the host-side output bias: sum_t p[s,t] (v_t + vb)
    = (sum_t p v) + vb since sum p = 1, so the host adds vb @ ow.T + ob.

Per-core kernel, software-pipelined so ScalarE's exp (the #2 engine load)
overlaps projection matmuls. Emission order is engineered around the Tile
framework's conservative ordinal-based cross-engine waits: a consumer
waits on the producer ENGINE's instruction counter at its emission point,
so anything emitted right after X on another engine effectively waits for
X. Hence:
  - per attention iteration j: QK(j) -> one deferred-work filler ->
    PV(j-3) -> exp(j); exp is 3 iterations ahead of PV and never in the
    same-iteration chain; the 3-slot score ring keeps QK's slot-reuse WAR
    (vs exp(j-3)) off the critical path.
  - the causal mask of diagonal 128-blocks is ADDED BY THE PE itself
    (one accumulating matmul ident.T @ strict_lower(-30000) into the
    scores PSUM) -- no Pool/DVE hop between QK and exp.
  - normalize (reciprocal on DVE -> partition_broadcast on Pool ->
    multiply on DVE) has its recips inline at chunk end and the
    bcast+mul tail deferred into the next chunk's filler slots; output
    projection blocks are likewise deferred fillers, and the last two
    projection chunks defer their K/V projections into the exp-bound
    attention tail (fillers_hi).
  - Q/K projections transposed [feat 128, seq]; V computed directly in
    [t, feat] layout (no PE transposes), with V_aug column 64 = 1.0 so
    the PV matmul also produces the softmax denominator.
  - PSUM drains only on DVE/Act (Pool/GPSIMD cannot access PSUM on TRN2):
    Q bias-add + output drains + normalize on DVE, K/V copies on Act.
  - per-body DRAM traffic: x loaded once (fp16, chunked, both DMA queues,
    weights first on the Act queue), output stored as one 1MB DMA per
    4-row-block group on the SP queue.
  - with repeat>1 (timing NEFFs), qT/kT/oT/V_aug are double-buffered per
    body so consecutive bodies pipeline.

PSUM plan (8 banks): sc ring 3x2 banks (scores + q/k/v projection
accumulators + output-projection tiles) + po 2 banks (PV accumulator,
row 64 = denominator).
"""

import numpy as np

import concourse.bass as bass
import concourse.mybir as mybir
import concourse.tile as tile
from concourse import bacc
from concourse.bass import ts
from concourse.bass_utils import run_bass_kernel_spmd
from concourse.masks import make_identity, make_lower_triangular

B, S, D, H = 2, 2048, 1024, 16
HD = D // H            # 64 head dim
NCORES = 8
FPC = D // NCORES      # 128 features per core
HPC = FPC // HD        # 2 heads per core
P = 128
SQ = 512               # query chunk (matmul free dim)
NSQ = S // SQ          # 4
NTB = S // P           # 16 t-blocks
DBLK = D // P          # 8 contraction blocks for projections

F32 = mybir.dt.float32
MM_DT = mybir.dt.float16

_module_cache = {}


def _build_module(repeat=1):
    nc = bacc.Bacc("TRN2", target_bir_lowering=False, debug=False)

    xT_d = nc.dram_tensor("xT", [B, D, S], MM_DT, kind="ExternalInput").ap()
    # weights arrive host-pre-rearranged to [P, DBLK, FPC] so the DMA rows
    # are contiguous 2KB per partition (no 256B gather descriptors)
    qwT_d = nc.dram_tensor("qwT", [P, DBLK, FPC], MM_DT, kind="ExternalInput").ap()
    kwT_d = nc.dram_tensor("kwT", [P, DBLK, FPC], MM_DT, kind="ExternalInput").ap()
    vwT_d = nc.dram_tensor("vwT", [P, DBLK, FPC], MM_DT, kind="ExternalInput").ap()
    qb_d = nc.dram_tensor("qb", [FPC, 1], F32, kind="ExternalInput").ap()
    owT_d = nc.dram_tensor("owT", [FPC, D], MM_DT, kind="ExternalInput").ap()
    out_d = nc.dram_tensor("out", [B, S, D], MM_DT, kind="ExternalOutput").ap()

    # [B, D, S] with D split into 8 blocks of 128 partitions
    xT_r = xT_d.rearrange("b (o p) s -> b p o s", p=P)

    with tile.TileContext(nc) as tc:
        with (
            tc.tile_pool(name="singles", bufs=1) as singles,
            tc.tile_pool(name="xin", bufs=8) as xin,
            tc.tile_pool(name="ptile", bufs=6) as ptile,
            tc.tile_pool(name="small", bufs=4) as small,
            tc.tile_pool(name="outsb", bufs=2) as outsb,
            tc.tile_pool(name="scp", bufs=3, space="PSUM") as scp,
            tc.tile_pool(name="pop", bufs=1, space="PSUM") as pop,
        ):
            # --- constants / persistent tensors ---
            qwT_sb = singles.tile([P, DBLK, FPC], MM_DT, tag="qw")
            kwT_sb = singles.tile([P, DBLK, FPC], MM_DT, tag="kw")
            vwT_sb = singles.tile([P, DBLK, FPC], MM_DT, tag="vw")
            # weights go out on the Activation-engine DMA queue so they load
            # in parallel with the x chunks on the SP queue
            nc.scalar.dma_start(out=qwT_sb, in_=qwT_d)
            nc.scalar.dma_start(out=kwT_sb, in_=kwT_d)
            nc.scalar.dma_start(out=vwT_sb, in_=vwT_d)
            qb_sb = singles.tile([FPC, 1], F32, tag="qb")
            nc.scalar.dma_start(out=qb_sb, in_=qb_d)
            owT_sb = singles.tile([FPC, D], MM_DT, tag="ow")
            nc.scalar.dma_start(out=owT_sb, in_=owT_d)

            # causal-mask-as-matmul constants: sc += ident.T @ umask adds
            # -30000 above the diagonal of a 128-block (keeps ScalarE/Pool
            # off the per-block critical path)
            ident = singles.tile([P, P], MM_DT, tag="ident")
            make_identity(nc, ident)
            umask = singles.tile([P, P], MM_DT, tag="umask")
            make_lower_triangular(nc, umask, val=-30000.0, diag=False)

            # double-buffered across repeat bodies: body r uses set r%2, so
            # body r+1's projections never WAR-serialize against body r's
            # attention reads
            nbuf = 2 if repeat > 1 else 1
            qT_sbs, kT_sbs, oT_sbs, v_augs = [], [], [], []
            ones_sb = singles.tile([P, 1], F32, tag="ones")
            nc.vector.memset(ones_sb, 1.0)
            for r in range(nbuf):
                qT_sbs.append(singles.tile([P, B, S], MM_DT, tag=f"qT{r}",
                                           name=f"qT{r}"))
                kT_sbs.append(singles.tile([P, B, S], MM_DT, tag=f"kT{r}",
                                           name=f"kT{r}"))
                oT_sbs.append(singles.tile([P, B, S], MM_DT, tag=f"oT{r}",
                                           name=f"oT{r}"))
                # V_aug[t, b, h, tblk, 0:64] = v features; [.., 64] = 1.0
                va = singles.tile([P, B, HPC, NTB, HD + 1], MM_DT,
                                  tag=f"vaug{r}", name=f"vaug{r}")
                nc.vector.tensor_copy(
                    out=va[:, :, :, :, HD],
                    in_=ones_sb[:, 0][:, None, None, None].to_broadcast(
                        [P, B, HPC, NTB]),
                )
                v_augs.append(va)

            # deferred-work queues persist across repeat bodies: the last
            # chunk's normalize/flush drains inside the NEXT body's loop
            # instead of stalling the inter-body boundary
            queues = {"out": [], "hi": [], "nm": []}
            env = dict(locals())
            for _rep in range(repeat):
                _emit_body(nc, tc, env, _rep % nbuf, _rep == repeat - 1)

    return nc


def _emit_body(nc, tc, env, bufi=0, last=True):
    g = type("G", (), env)
    singles, xin, ptile, small, outsb = g.singles, g.xin, g.ptile, g.small, g.outsb
    scp, pop = g.scp, g.pop
    qwT_sb, kwT_sb, vwT_sb = g.qwT_sb, g.kwT_sb, g.vwT_sb
    qb_sb, owT_sb = g.qb_sb, g.owT_sb
    qT_sb, kT_sb = g.qT_sbs[bufi], g.kT_sbs[bufi]
    oT_sb, v_aug = g.oT_sbs[bufi], g.v_augs[bufi]
    xT_r, out_d = g.xT_r, g.out_d
    ident, umask = g.ident, g.umask

    fillers = g.queues["out"]
    fillers_hi = g.queues["hi"]
    nmq = g.queues["nm"]
    # [b, (sb p), d] view of the output: 4 row-blocks go out as one DMA
    out_r = out_d.rearrange("b (i sb p) d -> b i p sb d", sb=4, p=P)

    def emit_outblock(b, i, s, ot4):
        pp = scp.tile([P, HPC, SQ], F32, tag="sc", name=f"pp{b}_{s}")
        for cc in range(2):
            nc.tensor.matmul(
                pp[:, cc, :],
                lhsT=oT_sb[:, b, ts(s, P)],
                rhs=owT_sb[:, ts(cc, SQ)],
                start=True,
                stop=True,
            )
        sl = s - 4 * i
        # PSUM drains are DVE/Act only on TRN2; DVE has the most slack
        nc.vector.tensor_copy(out=ot4[:, sl, :],
                              in_=pp.rearrange("p h c -> p (h c)"))
        if sl == 3:
            nc.sync.dma_start(out=out_r[b, i], in_=ot4)

    xt_tiles = {}

    def load_x(b, cn, split=False):
        xt = xin.tile([P, DBLK, SQ], MM_DT, tag="xt", name=f"xt{b}{cn}")
        if split:
            # first chunk: o-sliced on the SP queue so the o=0 projection
            # matmul can start ~0.5us in (weights load on the Act queue
            # in parallel)
            for o in range(DBLK):
                nc.sync.dma_start(out=xt[:, o, :], in_=xT_r[b, :, o, ts(cn, SQ)])
        else:
            eng = nc.sync if cn % 2 == 0 else nc.scalar
            eng.dma_start(out=xt, in_=xT_r[b, :, :, ts(cn, SQ)])
        xt_tiles[(b, cn)] = xt

    def proj_q(b, cn, xt):
        pq = scp.tile([P, SQ], F32, tag="sc", name=f"pq{b}{cn}")
        for o in range(DBLK):
            nc.tensor.matmul(
                pq, lhsT=qwT_sb[:, o, :], rhs=xt[:, o, :],
                start=(o == 0), stop=(o == DBLK - 1),
            )
        nc.vector.tensor_scalar_add(
            out=qT_sb[:, b, ts(cn, SQ)], in0=pq, scalar1=qb_sb,
        )

    def proj_k(b, cn, xt):
        pk = scp.tile([P, SQ], F32, tag="sc", name=f"pk{b}{cn}")
        for o in range(DBLK):
            nc.tensor.matmul(
                pk, lhsT=kwT_sb[:, o, :], rhs=xt[:, o, :],
                start=(o == 0), stop=(o == DBLK - 1),
            )
        nc.scalar.copy(out=kT_sb[:, b, ts(cn, SQ)], in_=pk)

    def proj_v(b, cn, xt):
        # V projection, direct [t, feat] layout: 4 t-blocks in one bank
        pv = scp.tile([P, 4, P], F32, tag="sc", name=f"pv{b}{cn}")
        for tb in range(4):
            for o in range(DBLK):
                nc.tensor.matmul(
                    pv[:, tb, :],
                    lhsT=xt[:, o, ts(tb, P)],
                    rhs=vwT_sb[:, o, :],
                    start=(o == 0),
                    stop=(o == DBLK - 1),
                    skip_group_check=True,
                )
        nc.scalar.copy(
            out=v_aug[:, b, :, ts(cn, 4), 0:HD],
            in_=pv.rearrange("p t (h d) -> p h t d", h=HPC),
        )

    def ph1(b, cn, parts="qkv"):
        xt = xt_tiles[(b, cn)]
        if "q" in parts:
            proj_q(b, cn, xt)
        if "k" in parts:
            proj_k(b, cn, xt)
        if "v" in parts:
            proj_v(b, cn, xt)

    def att(b, i):
        po = pop.tile([HD + 1, HPC, SQ], F32, tag="po", name=f"po{b}_{i}")
        jmax = 4 * i + 3

        def col0_of(j):
            k = j - 4 * i
            return P * k if k > 0 else 0

        def emit_qk(j):
            col0 = col0_of(j)
            diag = j - 4 * i >= 0
            sc = scp.tile([P, HPC, SQ], F32, tag="sc", name=f"sc{b}{i}{j}")
            for h in range(HPC):
                hs = h * HD
                nc.tensor.matmul(
                    sc[:, h, col0:],
                    lhsT=kT_sb[hs:hs + HD, b, ts(j, P)],
                    rhs=qT_sb[hs:hs + HD, b, i * SQ + col0:(i + 1) * SQ],
                    start=True,
                    stop=not diag,
                    skip_group_check=diag,
                )
                if diag:
                    # accumulate the additive causal mask for the diagonal
                    # 128-block via the PE (no Pool/DVE hop before exp)
                    nc.tensor.matmul(
                        sc[:, h, col0:col0 + P],
                        lhsT=ident,
                        rhs=umask,
                        start=False,
                        stop=True,
                        skip_group_check=True,
                    )
            return sc

        # 2-deep software pipeline, ordered for the framework's
        # conservative ordinal-based cross-engine waits: per iteration emit
        # QK(j), filler, PV(j-2), exp(j) — exp sits right after PE work
        # that is off the exp->PV serial chain, and PV(j-2)'s exp finished
        # two iterations ago. The 3-slot sc ring keeps QK(j)'s slot-reuse
        # WAR (vs exp(j-3)) off the critical path.
        def emit_exp(j, sc):
            col0 = col0_of(j)
            pt = ptile.tile([P, HPC, SQ], MM_DT, tag="pt", name=f"pt{b}{i}{j}")
            nc.scalar.activation(
                out=pt[:, :, col0:], in_=sc[:, :, col0:],
                func=mybir.ActivationFunctionType.Exp,
                scale=0.125,
            )
            return pt

        def emit_pv(j, pt):
            col0 = col0_of(j)
            for h in range(HPC):
                nc.tensor.matmul(
                    po[:, h, col0:],
                    lhsT=v_aug[:, b, h, j, :],
                    rhs=pt[:, h, col0:],
                    start=(j == 0),
                    stop=(j == jmax),
                    skip_group_check=True,
                )

        pt_q = []
        LAG = 3  # PV trails exp by 3 iterations (covers normalize latency
        # of the single-buffered po at chunk boundaries)
        for j in range(jmax + 1):
            sc = emit_qk(j)
            # deferred work from previous chunks, staged by dependency:
            # normalize at j=1 (no PE instructions, frees the po slot
            # ASAP), independent k/v projections from j>=1 (PE work with
            # no pending deps), output-projection blocks only from j>=4 —
            # by then the normalize their oT input needs has completed, so
            # they never head-of-line-block the PE queue.
            if j == 1:
                while nmq:
                    nmq.pop(0)()
            if j >= 1 and fillers_hi:
                fillers_hi.pop(0)()
            elif j >= 4 and fillers:
                fillers.pop(0)()
            if j >= LAG:
                emit_pv(j - LAG, pt_q.pop(0))
            pt_q.append(emit_exp(j, sc))
        for j in range(max(0, jmax - LAG + 1), jmax + 1):
            emit_pv(j, pt_q.pop(0))
        # normalize: oT = po[0:64] * (1 / denom), denom = po[64].
        # recips run inline (cheap, start right after the last PV); the
        # bcast+mul tail is deferred into the next chunk's filler slots.
        rcs = []
        for h in range(HPC):
            rc = small.tile([1, SQ], F32, tag="rc", name=f"rc{b}{i}{h}")
            nc.vector.reciprocal(out=rc, in_=po[HD:HD + 1, h, :])
            rcs.append(rc)

        def normalize(b=b, i=i, po=po, rcs=rcs):
            sq = ts(i, SQ)
            rbs = []
            for h in range(HPC):
                rb = small.tile([HD, SQ], F32, tag="rb", name=f"rb{b}{i}{h}")
                nc.gpsimd.partition_broadcast(out_ap=rb, in_ap=rcs[h])
                rbs.append(rb)
            for h in range(HPC):
                hs = h * HD
                nc.vector.tensor_mul(
                    out=oT_sb[hs:hs + HD, b, sq], in0=po[0:HD, h, :],
                    in1=rbs[h],
                )

        ot4 = outsb.tile([P, 4, D], MM_DT, tag="ot", name=f"ot{b}_{i}")
        nmq.append(normalize)
        fillers.extend(
            (lambda s=s, b=b, i=i: emit_outblock(b, i, s, ot4))
            for s in range(4 * i, 4 * i + 4)
        )

    # software pipeline: attention chunk (b,i) needs ph1 chunks (b,0..i);
    # interleave so exp overlaps projection matmuls throughout. The last
    # two projection chunks run q-only in their stage slot; their k/v
    # projections are deferred into the preceding attention chunk's filler
    # queue — PE work moved into the exp-bound tail.
    stages = [
        ("p", 0, 0, "qkv"), ("p", 0, 1, "qkv"), ("a", 0, 0), ("p", 0, 2, "qkv"),
        ("a", 0, 1), ("p", 0, 3, "qkv"), ("a", 0, 2), ("p", 1, 0, "qkv"),
        ("a", 0, 3), ("p", 1, 1, "qkv"), ("a", 1, 0), ("p", 1, 2, "q"),
        ("a", 1, 1), ("p", 1, 3, "q"), ("a", 1, 2), ("a", 1, 3),
    ]
    # issue every x-chunk DMA upfront so none queues behind the out DMAs
    # that start mid-kernel
    first = True
    for st in stages:
        if st[0] == "p":
            load_x(st[1], st[2], split=first)
            first = False
    for st in stages:
        if st[0] == "p":
            _, b, i, parts = st
            ph1(b, i, parts)
            if parts == "q":
                xt = xt_tiles[(b, i)]
                fillers_hi.append(lambda b=b, i=i, xt=xt: proj_k(b, i, xt))
                fillers_hi.append(lambda b=b, i=i, xt=xt: proj_v(b, i, xt))
        else:
            att(st[1], st[2])
    if last:
        while nmq:
            nmq.pop(0)()
        while fillers_hi:
            fillers_hi.pop(0)()
        while fillers:
            fillers.pop(0)()


def get_module(repeat=1):
    key = ("nc", repeat)
    if key not in _module_cache:
        m = _build_module(repeat=repeat)
        m.compile()
        _module_cache[key] = m
    return _module_cache[key]


def make_in_maps(x, qw, qb, kw, kb, vw, vb, ow):
    mmdt = np.dtype(np.float16)
    xT = np.ascontiguousarray(x.transpose(0, 2, 1)).astype(mmdt)  # [B, D, S]
    in_maps = []
    for c in range(NCORES):
        sl = slice(c * FPC, (c + 1) * FPC)
        def warr(w):
            # [D, FPC] -> [P, DBLK, FPC]: partition p owns D-rows p, p+128, ...
            wt = w[sl, :].T.reshape(DBLK, P, FPC)
            return np.ascontiguousarray(wt.transpose(1, 0, 2)).astype(mmdt)

        m = {
            "xT": xT,
            "qwT": warr(qw),
            "kwT": warr(kw),
            "vwT": warr(vw),
            "qb": np.ascontiguousarray(qb[sl].reshape(FPC, 1)).astype(np.float32),
            "owT": np.ascontiguousarray(ow[:, sl].T).astype(mmdt),
        }
        in_maps.append(m)
    return in_maps


def kernel(x, qw, qb, kw, kb, vw, vb, ow, ob, _trace=False):
    x = np.asarray(x, dtype=np.float32)
    qw = np.asarray(qw, dtype=np.float32)
    qb = np.asarray(qb, dtype=np.float32)
    kw = np.asarray(kw, dtype=np.float32)
    kb = np.asarray(kb, dtype=np.float32)
    vw = np.asarray(vw, dtype=np.float32)
    vb = np.asarray(vb, dtype=np.float32)
    ow = np.asarray(ow, dtype=np.float32)
    ob = np.asarray(ob, dtype=np.float32)

    nc = get_module()
    in_maps = make_in_maps(x, qw, qb, kw, kb, vw, vb, ow)
    res = run_bass_kernel_spmd(
        nc, in_maps, core_ids=list(range(NCORES)), trace=_trace
    )
    acc = np.zeros((B, S, D), dtype=np.float64)
    for r in res.results:
        acc += r["out"].astype(np.float64)
    # host-side bias: ob + vb @ ow.T (V bias folded out of the device kernel)
    bias = ob.astype(np.float64) + vb.astype(np.float64) @ ow.astype(np.float64).T
    out = (acc + bias).astype(np.float32)
    if _trace:
        kernel.last_results = res
    return out
